# revision 15
# baseline (speedup 1.0000x reference)
"""Trainium2 Bass kernel for nn_BaselineGCN (2-layer GCN + BN + mean-pool + MLP head).

Strategy (8 NeuronCores, dst-sharded nodes):
 - All scaling (gcn_norm dinv factors, edge weights, BN scale) is folded host-side
   into fp8 one-hot matrices B[slot, dst] streamed from DRAM; BN scale folds into
   the weight matrices (relu(S*x)=S*relu(x)), BN bias enters the PSUM accumulation
   as a K=1 rank-1 matmul. The per-block epilogue is a single ScalarE Relu
   (PSUM -> SBUF bf16).
 - Layer-0 projection x@W0' is computed REPLICATED on every core from a
   host-pre-transposed fp8 xT (no AllGather, no on-device transposes); each core
   writes the full bf16 gather table for layer 0 into its local DRAM.
 - Table rows live in a (p*8+i)-permuted order so projection writes are
   2KB-per-partition contiguous DMAs.
 - Layer-0 aggregation runs "flipped" (lhsT=gathered tile, rhs=B) producing
   feature-major y0^T, which feeds the layer-1 projection without a transpose.
   Layer-1 aggregation runs node-major; pooling accumulates inline via fp8
   one-hot batch tiles. Self-loops are K=128 diag matmuls against locally
   recomputed (L0) / retained (L1) own-block projection tiles.
 - The per-edge gather h'[src] uses SWDGE dma_gather (256B rows) over 4 queues
   with a 2x-deep descriptor ring (dynamic_dma_scratch_size=32768).
 - Layer-1 table is AllGather'ed in 4 quarter-window chunks overlapping layer-0.
"""
import sys

sys.path.insert(0, "/opt/trn_rl_repo")

import numpy as np

P = 128
HID = 64
NCLASS = 10
HHID = 32
G = 128
N = 100000
NCORES = 8
NSH = N // NCORES          # 12500
NB = 98                    # blocks per core (97*128 + 84)
LB = NSH - (NB - 1) * P    # 84
EPS = 1e-5

QN = [3072, 3072, 3072, 3284]    # real nodes per quarter per core
QP = [3072, 3072, 3072, 4096]    # padded stripe rows per quarter
QB0 = [0, 24, 48, 72]            # first block of each quarter
QN0 = [0, 3072, 6144, 9216]
WW = [8 * q for q in QP]          # window sizes [24576,24576,24576,32768]
WS = [0, 24576, 49152, 73728]
NT = sum(WW)                      # 106496 table rows
NUNIT = [q // 1024 for q in QP]   # units per stripe [3,3,3,4]
NUTOT = sum(NUNIT)                # 13 units per core shard
UIDX0 = [0, 3, 6, 9]              # unit index base per quarter

MAXCALL = 1024
NQUEUES = 4
SCRATCH = 32768
BT16 = 32                # B-stream chunks per DMA
GBUFS = 12
BBUFS = 3


def _ceil(a, b):
    return -(-a // b)


# ---------------------------------------------------------------- host prep --

def _dramrow(n):
    """node -> (storage table row, quarter)."""
    c = n // NSH
    r = n % NSH
    k = np.digitize(r, [3072, 6144, 9216])
    j = r - np.take(QN0, k)
    u = j // 1024
    rem = j % 1024
    i = rem // 128
    p = rem % 128
    return np.take(WS, k) + c * np.take(QP, k) + u * 1024 + p * 8 + i, k


def _gpos(n):
    """node -> xT column (processing order)."""
    c = n // NSH
    r = n % NSH
    k = np.digitize(r, [3072, 6144, 9216])
    j = r - np.take(QN0, k)
    return np.take(WS, k) + c * np.take(QP, k) + j


class Prep:
    def __init__(self, x, edge_index, batch, edge_attr, params):
        import ml_dtypes
        f8 = ml_dtypes.float8_e4m3
        bf = ml_dtypes.bfloat16

        src = np.asarray(edge_index[0], np.int64)
        dst = np.asarray(edge_index[1], np.int64)
        ew = np.asarray(edge_attr, np.float32)
        batch = np.asarray(batch, np.int64)
        x = np.asarray(x, np.float32)

        deg = np.bincount(dst, weights=ew.astype(np.float64), minlength=N) + 1.0
        dinv = (1.0 / np.sqrt(deg)).astype(np.float32)
        norm = (dinv[src] * ew * dinv[dst]).astype(np.float32)

        srow, _ = _dramrow(src)
        w = np.digitize(srow, WS[1:])
        loc = (srow - np.take(WS, w)).astype(np.int64)
        core = dst // NSH
        rb = dst % NSH
        b = rb // P
        dl = rb % P

        counts = np.zeros((NCORES, NB, 4), np.int64)
        np.add.at(counts, (core, b, w), 1)
        cap = counts.max(axis=0)
        cap = _ceil(np.maximum(cap, 0), P) * P  # [NB, 4]
        self.cap = cap

        sgs = []
        cur = [0]
        for bb in range(1, NB):
            trial = cur + [bb]
            if all(cap[trial, wi].sum() <= MAXCALL for wi in range(4)):
                cur = trial
            else:
                sgs.append(cur)
                cur = [bb]
        sgs.append(cur)
        self.sgs = sgs

        self.gcols = {}
        self.icol = {}
        self.coloff = {}
        sid = 0
        for sgi, sg in enumerate(sgs):
            for wi in range(4):
                cols = int(cap[sg, wi].sum()) // P
                self.gcols[(sgi, wi)] = cols
                self.icol[(sgi, wi)] = sid
                off = 0
                for bb in sg:
                    self.coloff[(bb, wi)] = off
                    off += int(cap[bb, wi]) // P
                sid += cols * 8
        self.SID = max(sid, 8)
        self.CT = max(int(cap.sum()) // P, 1)
        self.GMAX = max(max(self.gcols.values(), default=1), 1)
        self.nch = [int(cap[bb].sum()) // P for bb in range(NB)]

        order = np.lexsort((loc, w, b, core))
        self.src_loc = loc[order]
        self.dst_dl = dl[order]
        self.norm_s = norm[order]
        cum = np.zeros(NCORES * NB * 4 + 1, np.int64)
        key = (core * NB + b) * 4 + w
        np.cumsum(np.bincount(key, minlength=NCORES * NB * 4), out=cum[1:])
        self.grp = cum

        def fold(Wm, g, be, m, v, bias):
            S = (np.asarray(g) / np.sqrt(np.asarray(v) + EPS)).astype(np.float32)
            C = ((np.asarray(bias) - np.asarray(m)) * S + np.asarray(be)).astype(np.float32)
            return (np.asarray(Wm) * S[None, :]).astype(np.float32), C

        w0p, c0 = fold(params["W0"], params["g0"], params["be0"], params["m0"], params["v0"], params["b0"])
        w1p, c1 = fold(params["W1"], params["g1"], params["be1"], params["m1"], params["v1"], params["b1"])
        wf1p, cf = fold(params["Wf1"], params["gf"], params["bef"], params["mf"], params["vf"], params["bf1"])

        self.f8, self.bf = f8, bf
        self.dinv = dinv
        self.batch = batch
        self.w0p = w0p.astype(bf)
        self.w1p = w1p.astype(bf)
        self.c0row = np.ascontiguousarray(c0[None, :]).astype(bf)
        self.c1row = np.ascontiguousarray(c1[None, :]).astype(bf)
        self.wf1p = wf1p
        self.cfb = np.tile(cf[None, :], (P, 1)).astype(np.float32)
        self.wf2 = np.asarray(params["Wf2"], np.float32)
        self.bf2b = np.tile(np.asarray(params["bf2"], np.float32)[None, :], (P, 1))
        cnt = np.bincount(batch, minlength=G).astype(np.float32)
        self.invcnt = (1.0 / np.maximum(cnt, 1.0)).astype(np.float32)[:, None]
        self.ident = np.eye(P, dtype=np.float32)
        self.ones1 = np.ones((1, P), bf)

        gp = _gpos(np.arange(N))
        xTf = np.zeros((NT, P), np.float32)
        xTf[gp] = x
        self.xT = np.ascontiguousarray(xTf.T).astype(f8)
        self._xTf = xTf  # for per-core xoT slicing

    def core_arrays(self, c):
        f8 = self.f8
        idx_cols = np.zeros((P, self.SID), np.int16)
        Bf = np.zeros((self.CT, P, P), f8)
        t = 0
        for sgi, sg in enumerate(self.sgs):
            for wi in range(4):
                col = self.icol[(sgi, wi)]
                parts = []
                for bb in sg:
                    g0 = self.grp[(c * NB + bb) * 4 + wi]
                    g1 = self.grp[(c * NB + bb) * 4 + wi + 1]
                    locs = self.src_loc[g0:g1].astype(np.int16)
                    pad = int(self.cap[bb, wi]) - (g1 - g0)
                    parts.append(np.concatenate([locs, np.zeros(pad, np.int16)]))
                if parts:
                    flat = np.concatenate(parts)
                    if flat.size:
                        wrapped = np.tile(flat.reshape(-1, 16).T, (8, 1))
                        idx_cols[:, col:col + flat.size // 16] = wrapped
            for bb in sg:
                for wi in range(4):
                    g0 = self.grp[(c * NB + bb) * 4 + wi]
                    g1 = self.grp[(c * NB + bb) * 4 + wi + 1]
                    n = g1 - g0
                    capbw = int(self.cap[bb, wi])
                    if capbw == 0:
                        continue
                    if n:
                        sl = np.arange(n)
                        Bf[t + sl // P, sl % P, self.dst_dl[g0:g1]] = \
                            self.norm_s[g0:g1].astype(f8)
                    t += capbw // P
        B = np.ascontiguousarray(Bf.transpose(1, 0, 2))

        selfb = np.zeros((NB, P, P), f8)
        poolb = np.zeros((NB, P, P), f8)
        nodes0 = c * NSH
        d2 = (self.dinv[nodes0:nodes0 + NSH] ** 2).astype(f8)
        bt = self.batch[nodes0:nodes0 + NSH]
        for bb in range(NB):
            nb = P if bb < NB - 1 else LB
            pr = np.arange(nb)
            selfb[bb, pr, pr] = d2[bb * P:bb * P + nb]
            poolb[bb, pr, bt[bb * P:bb * P + nb]] = np.float32(1.0).astype(f8)
        selfb = np.ascontiguousarray(selfb.transpose(1, 0, 2))
        poolb = np.ascontiguousarray(poolb.transpose(1, 0, 2))

        # xoT: this core's shard in processing order, 13 units x 1024 cols
        cols = []
        for k in range(4):
            base = WS[k] + c * QP[k]
            cols.append(self._xTf[base:base + QP[k]])
        xoT = np.concatenate(cols, axis=0)  # [13*1024, 128]
        xoT = np.ascontiguousarray(xoT.T).astype(f8)
        return idx_cols, B, selfb, poolb, xoT


def _host_prep(x, edge_index, batch, edge_attr, params):
    pp = Prep(x, edge_index, batch, edge_attr, params)
    in_maps = []
    for c in range(NCORES):
        idx_cols, B, selfb, poolb, xoT = pp.core_arrays(c)
        in_maps.append(dict(
            xT=pp.xT, xoT=xoT, idxs=idx_cols, Bt=B, selfb=selfb, poolb=poolb,
            w0p=pp.w0p, w1p=pp.w1p, c0row=pp.c0row, c1row=pp.c1row,
            ones1=pp.ones1, wf1p=pp.wf1p, cfb=pp.cfb, wf2=pp.wf2,
            bf2b=pp.bf2b, invcnt=pp.invcnt, ident=pp.ident,
        ))
    return pp, in_maps


# ------------------------------------------------------------- bass program --

def build_nc(pp, reps=1):
    from concourse import mybir, bacc, tile

    f32 = mybir.dt.float32
    bf16 = mybir.dt.bfloat16
    i16 = mybir.dt.int16
    fp8 = mybir.dt.float8e4
    Alu = mybir.AluOpType
    Act = mybir.ActivationFunctionType

    CT, SID, GMAX = pp.CT, pp.SID, pp.GMAX
    sgs, cap, nch = pp.sgs, pp.cap, pp.nch

    nc = bacc.Bacc("TRN2", target_bir_lowering=False, debug=False,
                   enable_asserts=True, num_devices=NCORES,
                   num_swdge_queues=NQUEUES,
                   dynamic_dma_scratch_size=SCRATCH)

    I = {}
    def inp(name, shape, dt=f32):
        I[name] = nc.dram_tensor(name, shape, dt, kind="ExternalInput")
        return I[name]

    inp("xT", [P, NT], fp8)
    inp("xoT", [P, NUTOT * 1024], fp8)
    inp("idxs", [P, SID], i16)
    inp("Bt", [P, CT, P], fp8)
    inp("selfb", [P, NB, P], fp8)
    inp("poolb", [P, NB, P], fp8)
    inp("w0p", [P, HID], bf16)
    inp("w1p", [HID, HID], bf16)
    inp("c0row", [1, HID], bf16)
    inp("c1row", [1, HID], bf16)
    inp("ones1", [1, P], bf16)
    inp("wf1p", [HID, HHID])
    inp("cfb", [P, HHID])
    inp("wf2", [HHID, NCLASS])
    inp("bf2b", [P, NCLASS])
    inp("invcnt", [P, 1])
    inp("ident", [P, P])
    out_d = nc.dram_tensor("out", [G, NCLASS], f32, kind="ExternalOutput")

    qctr = [0]
    def next_q():
        q = qctr[0] % NQUEUES
        qctr[0] += 1
        return q

    def quarter(bb):
        return 0 if bb < 24 else (1 if bb < 48 else (2 if bb < 72 else 3))

    with tile.TileContext(nc) as tc:
        import contextlib
        with contextlib.ExitStack() as ctx:
            const = ctx.enter_context(tc.tile_pool(name="const", bufs=1))
            stream = ctx.enter_context(tc.tile_pool(name="stream", bufs=1))
            xtp = ctx.enter_context(tc.tile_pool(name="xtp", bufs=3))
            padp = ctx.enter_context(tc.tile_pool(name="padp", bufs=3))
            gpool = ctx.enter_context(tc.tile_pool(name="gpool", bufs=GBUFS))
            bpool = ctx.enter_context(tc.tile_pool(name="bpool", bufs=BBUFS))
            sbp = ctx.enter_context(tc.tile_pool(name="sbp", bufs=3))
            sfxp = ctx.enter_context(tc.tile_pool(name="sfxp", bufs=3))
            h1up = ctx.enter_context(tc.tile_pool(name="h1up", bufs=NUTOT))
            ypool = ctx.enter_context(tc.tile_pool(name="ypool", bufs=4))
            tmp = ctx.enter_context(tc.tile_pool(name="tmp", bufs=6))
            pstrip = ctx.enter_context(tc.tile_pool(name="pstrip", bufs=2, space="PSUM"))
            pacc = ctx.enter_context(tc.tile_pool(name="pacc", bufs=4, space="PSUM"))
            ppool = ctx.enter_context(tc.tile_pool(name="ppool", bufs=1, space="PSUM"))
            phw = ctx.enter_context(tc.tile_pool(name="phw", bufs=1, space="PSUM"))
            dram = ctx.enter_context(tc.tile_pool(name="dram", bufs=1, space="DRAM"))

            C = {}
            for nm in ["w0p", "w1p", "c0row", "c1row", "ones1", "wf1p",
                       "cfb", "wf2", "bf2b", "invcnt", "ident"]:
                t_ = const.tile(list(I[nm].shape), I[nm].dtype, tag=nm)
                nc.sync.dma_start(out=t_[:], in_=I[nm][:])
                C[nm] = t_
            idx_t = stream.tile([P, SID], i16, tag="idx")
            nc.sync.dma_start(out=idx_t[:], in_=I["idxs"][:])

            shspace = "Shared" if reps == 1 else "Local"
            table0 = [dram.tile([WW[k], P], bf16, name=f"t0_{k}", tag=f"t0_{k}") for k in range(4)]
            table1 = [dram.tile([WW[k], P], bf16, name=f"t1_{k}", tag=f"t1_{k}", addr_space=shspace)
                      for k in range(4)]
            bounce = [dram.tile([QP[k], P], bf16, name=f"bn_{k}", tag=f"bn_{k}") for k in range(4)]
            ar_in = dram.tile([G, HID], f32, tag="arin")
            ar_out = dram.tile([G, HID], f32, tag="arout", addr_space=shspace)

            for _rep in range(reps):
                # ---- phase A: table0 = x @ W0' (replicated; units processed
                # in pairs to halve the DMA count; xt loads issue on ACT's
                # HWDGE, table writes on SP's)
                ec = 0
                for k in range(4):
                    for c in range(NCORES):
                        u = 0
                        while u < NUNIT[k]:
                            q = min(2, NUNIT[k] - u)
                            base = c * QP[k] + u * 1024
                            gbase = WS[k] + base
                            xt = xtp.tile([P, 2048], fp8, tag="xt")
                            nc.scalar.dma_start(out=xt[:, :q * 1024],
                                                in_=I["xT"][:, gbase:gbase + q * 1024])
                            pad = padp.tile([P, 16, P], bf16, tag="pd")
                            for qq in range(q):
                                strip = pstrip.tile([P, 8, HID], f32, tag="st")
                                for i in range(8):
                                    nc.tensor.matmul(
                                        strip[:, i, :],
                                        lhsT=xt[:, qq * 1024 + i * P:qq * 1024 + (i + 1) * P],
                                        rhs=C["w0p"][:], start=True, stop=True)
                                if ec % 2 == 0:
                                    nc.vector.tensor_copy(
                                        out=pad[:, qq * 8:(qq + 1) * 8, 0:HID],
                                        in_=strip[:])
                                else:
                                    nc.scalar.activation(
                                        out=pad[:, qq * 8:(qq + 1) * 8, 0:HID],
                                        in_=strip[:], func=Act.Copy)
                                ec += 1
                            dst_ap = table0[k][base:base + q * 1024, :].rearrange(
                                "(q p i) f -> p q i f", q=q, p=P, i=8)
                            src_ap = pad[:, :q * 8, :].rearrange(
                                "p (q i) f -> p q i f", q=q, i=8)
                            nc.sync.dma_start(out=dst_ap, in_=src_ap)
                            u += q

                # ---- GCN layers
                h1units = [None] * NUTOT     # written in L0, read in L1
                for l in range(2):
                    table = table0 if l == 0 else table1
                    crow = C["c0row"] if l == 0 else C["c1row"]
                    t = 0
                    btile = None
                    bt_lo = bt_hi = 0
                    cur_unit = -1
                    sfx = None       # L0 self source [P, 8, HID] bf16
                    sbt = None       # selfb slice [P, <=8, P] fp8
                    pbt = None       # poolb slice (L1)
                    if l == 1:
                        pooled = ppool.tile([P, HID], f32, tag="pool")

                    for sgi, sg in enumerate(sgs):
                        gt = {}
                        for wi in range(4):
                            cols = pp.gcols[(sgi, wi)]
                            if cols == 0:
                                continue
                            gbf = gpool.tile([P, GMAX, P], bf16, tag="g")
                            ic = pp.icol[(sgi, wi)]
                            gt[wi] = gbf
                            nc.gpsimd.dma_gather(
                                out_ap=gbf[:, :cols, :],
                                in_ap=table[wi][:, :],
                                idxs_ap=idx_t[:, ic:ic + cols * 8],
                                num_idxs=cols * P,
                                num_idxs_reg=cols * P,
                                elem_size=P,
                                queue_num=next_q(),
                            )
                        for bb in sg:
                            k = quarter(bb)
                            lb_q = bb - QB0[k]
                            u = lb_q // 8
                            iu = lb_q % 8
                            uidx = UIDX0[k] + u
                            ublk = min(8, (QB0[k + 1] if k < 3 else NB) - (QB0[k] + u * 8))
                            nb = P if bb < NB - 1 else LB

                            if cur_unit != uidx:
                                cur_unit = uidx
                                b0 = QB0[k] + u * 8
                                sbt = sbp.tile([P, 8, P], fp8, tag="sb")
                                nc.scalar.dma_start(out=sbt[:, :ublk, :],
                                                    in_=I["selfb"][:, b0:b0 + ublk, :])
                                if l == 0:
                                    xo = xtp.tile([P, 1024], fp8, tag="xo")
                                    nc.scalar.dma_start(
                                        out=xo[:],
                                        in_=I["xoT"][:, uidx * 1024:(uidx + 1) * 1024])
                                    sstrip = pstrip.tile([P, 8, HID], f32, tag="st")
                                    for i in range(8):
                                        nc.tensor.matmul(sstrip[:, i, :],
                                                         lhsT=xo[:, i * P:(i + 1) * P],
                                                         rhs=C["w0p"][:],
                                                         start=True, stop=True)
                                    sfx = sfxp.tile([P, 8, HID], bf16, tag="sfx")
                                    nc.scalar.activation(out=sfx[:], in_=sstrip[:],
                                                         func=Act.Copy)
                                    h1units[uidx] = h1up.tile([P, 8, P], bf16,
                                                              name=f"h1u{uidx}", tag="h1u")
                                else:
                                    pbt = sbp.tile([P, 8, P], fp8, tag="pb")
                                    nc.scalar.dma_start(out=pbt[:, :ublk, :],
                                                        in_=I["poolb"][:, b0:b0 + ublk, :])

                            acc = pacc.tile([P, P], f32, tag="acc")
                            nmm = nch[bb] + 2
                            done = 0
                            for wi in range(4):
                                kk = int(cap[bb, wi]) // P
                                for j in range(kk):
                                    if t >= bt_hi:
                                        t0b = (t // BT16) * BT16
                                        hi = min(t0b + BT16, CT)
                                        btile = bpool.tile([P, BT16, P], fp8, tag="B")
                                        beng = nc.sync if (t0b // BT16) % 2 else nc.scalar
                                        beng.dma_start(out=btile[:, :hi - t0b, :],
                                                       in_=I["Bt"][:, t0b:hi, :])
                                        bt_lo, bt_hi = t0b, hi
                                    gsl = gt[wi][:, pp.coloff[(bb, wi)] + j, 0:HID]
                                    bsl = btile[:, t - bt_lo, :]
                                    if l == 0:
                                        nc.tensor.matmul(acc[0:HID, :], lhsT=gsl, rhs=bsl,
                                                         start=(done == 0), stop=False)
                                    else:
                                        nc.tensor.matmul(acc[:, 0:HID], lhsT=bsl, rhs=gsl,
                                                         start=(done == 0), stop=False)
                                    done += 1
                                    t += 1
                            # self chunk + bias chunk
                            if l == 0:
                                nc.tensor.matmul(acc[0:HID, :], lhsT=sfx[:, iu, :],
                                                 rhs=sbt[:, iu, :],
                                                 start=False, stop=False)
                                nc.tensor.matmul(acc[0:HID, :], lhsT=crow[:],
                                                 rhs=C["ones1"][:],
                                                 start=False, stop=True)
                                # epilogue: y0T = relu(accT)  [HID, 128] bf16
                                y0T = ypool.tile([P, P], bf16, tag="y0T")
                                nc.scalar.activation(out=y0T[0:HID, :],
                                                     in_=acc[0:HID, :], func=Act.Relu)
                                # projection h1 = y0 @ W1'  (K=HID)
                                h1ps = phw.tile([P, HID], f32, tag="h1p")
                                nc.tensor.matmul(h1ps[:], lhsT=y0T[0:HID, :],
                                                 rhs=C["w1p"][:], start=True, stop=True)
                                nc.scalar.activation(
                                    out=h1units[uidx][:, iu, 0:HID],
                                    in_=h1ps[:], func=Act.Copy)
                                if iu == ublk - 1:
                                    # flush unit to bounce stripe
                                    ub = u * 1024
                                    dst_ap = bounce[k][ub:ub + 1024, :].rearrange(
                                        "(p i) f -> p i f", p=P, i=8)
                                    nc.sync.dma_start(out=dst_ap, in_=h1units[uidx][:])
                                if bb == NB - 1 or (quarter(bb + 1) != k):
                                    nc.gpsimd.collective_compute(
                                        "AllGather", Alu.bypass,
                                        replica_groups=[list(range(NCORES))],
                                        ins=[bounce[k].opt()], outs=[table1[k].opt()],
                                    )
                            else:
                                nc.tensor.matmul(acc[:, 0:HID], lhsT=sbt[:, iu, :],
                                                 rhs=h1units[uidx][:, iu, 0:HID],
                                                 start=False, stop=False)
                                nc.tensor.matmul(acc[:, 0:HID], lhsT=C["ones1"][:],
                                                 rhs=crow[:], start=False, stop=True)
                                y1 = ypool.tile([P, HID], bf16, tag="y1")
                                nc.scalar.activation(out=y1[:], in_=acc[:, 0:HID],
                                                     func=Act.Relu)
                                nc.tensor.matmul(pooled[:], lhsT=pbt[:, iu, :], rhs=y1[:],
                                                 start=(bb == 0), stop=(bb == NB - 1))

                # ---- mean pool AllReduce + MLP head
                pl = tmp.tile([P, HID], f32, tag="pl")
                nc.vector.tensor_copy(out=pl[:G, :], in_=pooled[:G, :])
                nc.sync.dma_start(out=ar_in[:], in_=pl[:G, :])
                nc.gpsimd.collective_compute(
                    "AllReduce", Alu.add,
                    replica_groups=[list(range(NCORES))],
                    ins=[ar_in.opt()], outs=[ar_out.opt()],
                )
                pl2 = tmp.tile([P, HID], f32, tag="pl2")
                nc.sync.dma_start(out=pl2[:G, :], in_=ar_out[:])
                nc.vector.tensor_scalar(out=pl2[:G, :], in0=pl2[:G, :],
                                        scalar1=C["invcnt"][:G, :], scalar2=None,
                                        op0=Alu.mult)
                pt = pstrip.tile([P, 8, HID], f32, tag="st")
                ptv = pt[:].rearrange("p i f -> p (i f)")
                nc.tensor.transpose(ptv[0:HID, 0:G], pl2[:G, :], C["ident"][:])
                pts = tmp.tile([P, P], f32, tag="pts")
                nc.vector.tensor_copy(out=pts[0:HID, 0:G], in_=ptv[0:HID, 0:G])
                zp = phw.tile([P, HID], f32, tag="h1p")
                nc.tensor.matmul(zp[:G, 0:HHID], lhsT=pts[0:HID, 0:G], rhs=C["wf1p"][:],
                                 start=True, stop=True)
                z = tmp.tile([P, HHID], f32, tag="z")
                nc.vector.tensor_tensor(out=z[:G, :], in0=zp[:G, 0:HHID],
                                        in1=C["cfb"][:G, :], op=Alu.add)
                nc.vector.tensor_scalar(out=z[:G, :], in0=z[:G, :], scalar1=0.0,
                                        scalar2=None, op0=Alu.max)
                pt2 = pstrip.tile([P, 8, HID], f32, tag="st")
                ptv2 = pt2[:].rearrange("p i f -> p (i f)")
                nc.tensor.transpose(ptv2[0:HHID, 0:G], z[:G, :], C["ident"][:])
                zts = tmp.tile([P, P], f32, tag="pts")
                nc.vector.tensor_copy(out=zts[0:HHID, 0:G], in_=ptv2[0:HHID, 0:G])
                lp = phw.tile([P, HID], f32, tag="h1p")
                nc.tensor.matmul(lp[:G, 0:NCLASS], lhsT=zts[0:HHID, 0:G], rhs=C["wf2"][:],
                                 start=True, stop=True)
                lg = tmp.tile([P, NCLASS], f32, tag="lg")
                nc.vector.tensor_tensor(out=lg[:G, :], in0=lp[:G, 0:NCLASS],
                                        in1=C["bf2b"][:G, :], op=Alu.add)
                mx = tmp.tile([P, 1], f32, tag="mx")
                nc.vector.reduce_max(mx[:G, :], lg[:G, :], axis=mybir.AxisListType.X)
                nc.vector.tensor_scalar(out=lg[:G, :], in0=lg[:G, :], scalar1=mx[:G, :],
                                        scalar2=None, op0=Alu.subtract)
                ex = tmp.tile([P, NCLASS], f32, tag="ex")
                nc.scalar.activation(out=ex[:G, :], in_=lg[:G, :], func=Act.Exp)
                sm = tmp.tile([P, 1], f32, tag="sm")
                nc.vector.reduce_sum(sm[:G, :], ex[:G, :], axis=mybir.AxisListType.X)
                lsm = tmp.tile([P, 1], f32, tag="ls")
                nc.scalar.activation(out=lsm[:G, :], in_=sm[:G, :], func=Act.Ln)
                nc.vector.tensor_scalar(out=lg[:G, :], in0=lg[:G, :], scalar1=lsm[:G, :],
                                        scalar2=None, op0=Alu.subtract)
                nc.sync.dma_start(out=out_d[:], in_=lg[:G, :])

    nc.compile()
    return nc


# ------------------------------------------------------------ PJRT runner --

class SpmdRunner:
    """Run the compiled 8-core Bass module via PJRT (axon)."""

    def __init__(self, nc, n_cores):
        import jax
        from jax.sharding import Mesh, PartitionSpec
        from jax.experimental.shard_map import shard_map
        from concourse import bass2jax, mybir as _mb
        from concourse.bass2jax import _bass_exec_p, install_neuronx_cc_hook
        install_neuronx_cc_hook()
        self.jax = jax
        self.nc = nc
        self.n_cores = n_cores
        partition_name = nc.partition_id_tensor.name if nc.partition_id_tensor else None
        in_names, out_names, out_avals, zero_outs = [], [], [], []
        for alloc in nc.m.functions[0].allocations:
            if not isinstance(alloc, _mb.MemoryLocationSet):
                continue
            name = alloc.memorylocations[0].name
            if alloc.kind == "ExternalInput":
                if name != partition_name:
                    in_names.append(name)
            elif alloc.kind == "ExternalOutput":
                shape = tuple(alloc.tensor_shape)
                dtype = _mb.dt.np(alloc.dtype)
                out_names.append(name)
                out_avals.append(jax.core.ShapedArray(shape, dtype))
                zero_outs.append(np.zeros(shape, dtype))
        self.in_names, self.out_names = in_names, out_names
        self.out_avals, self.zero_outs = out_avals, zero_outs
        n_params, n_outs = len(in_names), len(out_avals)
        self.n_params = n_params
        all_in_names = in_names + out_names + ([partition_name] if partition_name else [])

        def _body(*args):
            operands = list(args)
            if partition_name is not None:
                operands.append(bass2jax.partition_id_tensor())
            return tuple(_bass_exec_p.bind(
                *operands, out_avals=tuple(out_avals), in_names=tuple(all_in_names),
                out_names=tuple(out_names), lowering_input_output_aliases=(),
                sim_require_finite=False, sim_require_nnan=False, nc=nc))

        devices = jax.devices()[:n_cores]
        assert len(devices) == n_cores
        mesh = Mesh(np.asarray(devices), ("core",))
        self._sharding = jax.sharding.NamedSharding(mesh, PartitionSpec("core"))
        in_specs = (PartitionSpec("core"),) * (n_params + n_outs)
        out_specs = (PartitionSpec("core"),) * len(out_names)
        self._fn = jax.jit(
            shard_map(_body, mesh=mesh, in_specs=in_specs,
                      out_specs=out_specs, check_rep=False),
            keep_unused=True)

    def prepare(self, in_maps):
        per_core = [[np.asarray(m[name]) for name in self.in_names] for m in in_maps]
        concat_in = [np.concatenate([per_core[c][i] for c in range(self.n_cores)], axis=0)
                     for i in range(self.n_params)]
        concat_zeros = [np.zeros((self.n_cores * z.shape[0], *z.shape[1:]), z.dtype)
                        for z in self.zero_outs]
        return concat_in + concat_zeros

    def run(self, in_maps):
        out_arrs = self._fn(*self.prepare(in_maps))
        self.jax.block_until_ready(out_arrs)
        return self._split(out_arrs)

    def _split(self, out_arrs):
        return [{name: np.asarray(out_arrs[i]).reshape(self.n_cores, *self.out_avals[i].shape)[c]
                 for i, name in enumerate(self.out_names)}
                for c in range(self.n_cores)]


# ------------------------------------------------------------------- driver --

_CACHE = {}
_LAST = {}


def kernel(**inputs):
    x = np.asarray(inputs["x"], np.float32)
    edge_index = np.asarray(inputs["edge_index"])
    batch = np.asarray(inputs["batch"])
    edge_attr = np.asarray(inputs["edge_attr"], np.float32)
    params = {k: np.asarray(v) for k, v in inputs.items()
              if k not in ("x", "edge_index", "batch", "edge_attr", "pos")}

    pp, in_maps = _host_prep(x, edge_index, batch, edge_attr, params)

    key = ("k2", x.shape, edge_index.shape, pp.SID, pp.CT, pp.GMAX,
           tuple(tuple(s) for s in pp.sgs))
    if key not in _CACHE:
        _CACHE[key] = SpmdRunner(build_nc(pp), NCORES)
    runner = _CACHE[key]
    _LAST.update(pp=pp, in_maps=in_maps, runner=runner)
    results = runner.run(in_maps)
    return results[0]["out"]


def estimate_exec_ns(reps=8, iters=10):
    """Per-execution device time via wall-clock delta between a 1-rep NEFF and
    an in-NEFF repeated body (cancels the axon dispatch floor)."""
    import time as _t
    import jax
    pp, in_maps, r1 = _LAST["pp"], _LAST["in_maps"], _LAST["runner"]
    rR = SpmdRunner(build_nc(pp, reps=reps), NCORES)
    a1 = [jax.device_put(a, r1._sharding) for a in r1.prepare(in_maps)]
    aR = [jax.device_put(a, rR._sharding) for a in rR.prepare(in_maps)]
    jax.block_until_ready(r1._fn(*a1)); jax.block_until_ready(rR._fn(*aR))
    t1s, tRs = [], []
    for _ in range(iters):
        t0 = _t.perf_counter(); jax.block_until_ready(r1._fn(*a1)); t1s.append(_t.perf_counter() - t0)
        t0 = _t.perf_counter(); jax.block_until_ready(rR._fn(*aR)); tRs.append(_t.perf_counter() - t0)
    t1s, tRs = sorted(t1s), sorted(tRs)
    per = (tRs[len(tRs) // 2] - t1s[len(t1s) // 2]) / (reps - 1)
    return per * 1e9
